# revision 6
# baseline (speedup 1.0000x reference)
"""Tacotron-style location-sensitive attention on 8 trn2 NeuronCores, v13.

Data-parallel over batch B=64 -> 8 batches per core; weights replicated.

v13: energies with t on partitions (interleave t = j*1024 + p*8 + n8),
eliminating the energies transpose entirely:
  1. Host folds conv_w+Wd into W2T, appends pq row; im2col ships as two
     T/2 halves stacked on the partition axis (balanced 128-part DMA).
  2. loc2: per 128-t chunk, lhsT = strided im2col view (63x128), rhs =
     [W2T; pq] (63x128) -> PSUM (128t x 128d), 4 chunks per bank.
  3. DVE adds pm (bf16) -> arg bf16; ACT tanh -> th bf16; DVE multiply
     by broadcast Wv and one 3D-AP reduce over d -> energies (128x16).
  4. ACT exp (+accumulated row sums) -> xr bf16, px; den via
     ones-matmul; context = accumulating PE matmuls of xr columns
     against interleaved mem tiles; ACT scales by 1/den.
"""

import numpy as np
import ml_dtypes

B, T = 64, 2048
RNN_DIM, EMB_DIM, ATT_DIM = 1024, 512, 128
N_FILT, KSIZE = 32, 31
PAD = (KSIZE - 1) // 2
NCORES = 8
BPC = B // NCORES
NCHUNK = T // 128   # 16
NQUAD = 4
QW = T // NQUAD     # 512
TH = T // 2         # 1024
K2 = 2 * KSIZE      # 62

_CACHE = {}


def _build_bass():
    import concourse.bacc as bacc
    import concourse.mybir as mybir
    import concourse.tile as tile
    from bass_rust import VecI64Pair
    from concourse._compat import get_trn_type

    fp32 = mybir.dt.float32
    bf16 = mybir.dt.bfloat16
    nc = bacc.Bacc(
        get_trn_type() or "TRN2",
        target_bir_lowering=False,
        debug=False,
        num_devices=NCORES,
    )

    im2d = nc.dram_tensor("im2d", (BPC, 128, TH), bf16, kind="ExternalInput")
    pmb = nc.dram_tensor("pmb", (BPC, T, ATT_DIM), bf16, kind="ExternalInput")
    mem = nc.dram_tensor("mem", (BPC, T, EMB_DIM), bf16, kind="ExternalInput")
    wvb = nc.dram_tensor("wvb", (128, T), bf16, kind="ExternalInput")
    w2pq = nc.dram_tensor("w2pq", (128, BPC * ATT_DIM), bf16, kind="ExternalInput")
    out = nc.dram_tensor("out", (BPC, EMB_DIM), fp32, kind="ExternalOutput")

    def ap_of(t, offset_elems, dims):
        a = t[:].copy()
        a.offset = offset_elems
        a.ap = VecI64Pair([list(d) for d in dims])
        return a

    AF = mybir.ActivationFunctionType

    with tile.TileContext(nc) as tc:
        with (
            tc.tile_pool(name="const", bufs=1) as constp,
            tc.tile_pool(name="pmq", bufs=5) as pmp,
            tc.tile_pool(name="icp", bufs=5) as icp,
            tc.tile_pool(name="memt", bufs=4) as memp,
            tc.tile_pool(name="argp", bufs=4) as argp,
            tc.tile_pool(name="thp", bufs=2) as thp,
            tc.tile_pool(name="mup", bufs=2) as mup,
            tc.tile_pool(name="enp", bufs=2) as enp,
            tc.tile_pool(name="xout", bufs=3) as xp,
            tc.tile_pool(name="res", bufs=2) as resp,
            tc.tile_pool(name="psL", bufs=2, space="PSUM") as psL,
            tc.tile_pool(name="psC", bufs=2, space="PSUM") as psC,
        ):
            ones128 = constp.tile([128, 1], fp32)
            nc.vector.memset(ones128[:], 1.0)
            w2pq_all = constp.tile([128, BPC * ATT_DIM], bf16)
            wvb_t = constp.tile([128, T], bf16)

            def dma_ic_pm(b):
                ic = icp.tile([128, TH], bf16, name="ic")
                nc.sync.dma_start(ic[:], im2d[b])
                pmt = pmp.tile([128, T], bf16, name="pmt")
                nc.sync.dma_start(
                    pmt[:],
                    ap_of(pmb, b * T * ATT_DIM,
                          [[1024, 128], [131072, 2], [1, 1024]]),
                )
                return ic, pmt

            def dma_mem(b):
                mt = memp.tile([128, NCHUNK * EMB_DIM], bf16, name="mt")
                nc.sync.dma_start(
                    mt[:],
                    ap_of(mem, b * T * EMB_DIM,
                          [[4096, 128], [524288, 2], [1, 4096]]),
                )
                return mt

            def energies(b, ic, pmt):
                th = thp.tile([128, T], bf16, name="th")
                for h in range(2):
                    base = 0 if h == 0 else 64
                    ic_r = ic[base : base + K2 + 1, :].rearrange(
                        "k (t s) -> k t s", s=8
                    )
                    w2 = w2pq_all[base : base + K2 + 1,
                                  b * ATT_DIM : (b + 1) * ATT_DIM]
                    lps = psL.tile([128, 2 * QW], fp32, name="lps")
                    for jj in range(8):
                        nc.tensor.matmul(
                            lps[:, jj * 128 : (jj + 1) * 128],
                            ic_r[:, :, jj], w2,
                            start=True, stop=True,
                        )
                    arg = argp.tile([128, 2 * QW], bf16, name="arg")
                    nc.vector.tensor_add(
                        arg[:], lps[:], pmt[:, h * 2 * QW : (h + 1) * 2 * QW]
                    )
                    nc.scalar.activation(
                        th[:, h * 2 * QW : (h + 1) * 2 * QW], arg[:], AF.Tanh
                    )
                mu = mup.tile([128, T], bf16, name="mu")
                nc.vector.tensor_mul(mu[:], th[:], wvb_t[:])
                en = enp.tile([128, NCHUNK], fp32, name="en")
                nc.vector.reduce_sum(
                    en[:].rearrange("p a -> p a ()"),
                    mu[:].rearrange("p (a b) -> p a b", a=NCHUNK),
                    axis=mybir.AxisListType.X,
                )
                xr = xp.tile([128, NCHUNK], bf16, tag="xr", name="xr")
                px = xp.tile([128, 1], fp32, tag="px", name="px")
                nc.scalar.activation(xr[:], en[:], AF.Exp, accum_out=px[:])
                return xr, px

            def context(b, xr, px, mt):
                den_ps = psC.tile([1, 1], fp32, tag="den", bufs=1, name="den_ps")
                nc.tensor.matmul(den_ps[:], ones128[:], px[:], start=True, stop=True)
                rec = resp.tile([1, 1], fp32, name="rec")
                nc.vector.reciprocal(rec[:], den_ps[:])
                ctx_ps = psC.tile([1, EMB_DIM], fp32, tag="ctx", name="ctx_ps")
                for n in range(NCHUNK):
                    nc.tensor.matmul(
                        ctx_ps[:],
                        xr[:, n : n + 1],
                        mt[:, n * EMB_DIM : (n + 1) * EMB_DIM],
                        start=(n == 0), stop=(n == NCHUNK - 1),
                    )
                ctx = resp.tile([1, EMB_DIM], fp32, name="ctx")
                nc.scalar.activation(ctx[:], ctx_ps[:], AF.Copy, scale=rec[:])
                nc.gpsimd.dma_start(out[b : b + 1, :], ctx[:])

            icpm = {0: dma_ic_pm(0)}
            nc.sync.dma_start(w2pq_all[:], w2pq[:, :])
            icpm[1] = dma_ic_pm(1)
            nc.sync.dma_start(wvb_t[:], wvb[:, :])
            mts = {0: dma_mem(0), 1: dma_mem(1)}
            xrpx = {}
            for i in range(BPC):
                if i + 2 < BPC:
                    icpm[i + 2] = dma_ic_pm(i + 2)
                if i + 2 < BPC:
                    mts[i + 2] = dma_mem(i + 2)
                xrpx[i] = energies(i, *icpm.pop(i))
                if i >= 1:
                    context(i - 1, *xrpx.pop(i - 1), mts.pop(i - 1))
            context(BPC - 1, *xrpx.pop(BPC - 1), mts.pop(BPC - 1))

    nc.compile()
    return nc


def build_in_maps(attention_hidden_state, memory, processed_memory,
                  attention_weights, attention_weights_cum,
                  Wq, conv_w, Wd, Wv, mask):
    f32 = np.float32
    bf = ml_dtypes.bfloat16
    ahs = np.asarray(attention_hidden_state, dtype=f32)
    pm = np.asarray(processed_memory, dtype=f32)
    aw = np.asarray(attention_weights, dtype=f32)
    awc = np.asarray(attention_weights_cum, dtype=f32)

    mem_bf = np.asarray(memory, dtype=f32).astype(bf)
    pm_bf = pm.astype(bf)
    pq = (ahs @ np.ascontiguousarray(np.asarray(Wq, f32).T)).astype(bf)
    W2 = np.asarray(Wd, f32) @ np.asarray(conv_w, f32).reshape(N_FILT, K2)
    W2T = np.ascontiguousarray(W2.T).astype(bf)
    wvb = np.ascontiguousarray(
        np.tile(np.asarray(Wv, f32).astype(bf)[None, :], (128, NCHUNK))
    )

    awpad = np.zeros((B, 2, T + 2 * PAD), np.float32)
    awpad[:, 0, PAD : PAD + T] = aw
    awpad[:, 1, PAD : PAD + T] = awc
    sb, sc, st = awpad.strides
    win = np.lib.stride_tricks.as_strided(
        awpad, (B, 2, KSIZE, T), (sb, sc, st, st)
    )
    im2col = win.reshape(B, K2, T)
    im2d = np.zeros((B, 128, TH), bf)
    im2d[:, 0:K2, :] = im2col[:, :, 0:TH].astype(bf)
    im2d[:, 62, :] = 1.0
    im2d[:, 64 : 64 + K2, :] = im2col[:, :, TH:T].astype(bf)
    im2d[:, 126, :] = 1.0

    in_maps = []
    for c in range(NCORES):
        s = slice(c * BPC, (c + 1) * BPC)
        w2pq_h = np.zeros((128, BPC * ATT_DIM), bf)
        for j, b in enumerate(range(c * BPC, (c + 1) * BPC)):
            blk = slice(j * ATT_DIM, (j + 1) * ATT_DIM)
            w2pq_h[0:K2, blk] = W2T
            w2pq_h[62, blk] = pq[b]
            w2pq_h[64 : 64 + K2, blk] = W2T
            w2pq_h[126, blk] = pq[b]
        in_maps.append({
            "im2d": np.ascontiguousarray(im2d[s]),
            "pmb": pm_bf[s],
            "mem": mem_bf[s],
            "wvb": wvb,
            "w2pq": w2pq_h,
        })
    return in_maps


def kernel(**inputs):
    from concourse.bass_utils import run_bass_kernel_spmd

    in_maps = build_in_maps(**inputs)
    if "nc" not in _CACHE:
        _CACHE["nc"] = _build_bass()
    nc = _CACHE["nc"]
    res = run_bass_kernel_spmd(nc, in_maps, core_ids=list(range(NCORES)))
    out = np.concatenate([r["out"] for r in res.results], axis=0)
    return out.astype(np.float32)
